# revision 1
# baseline (speedup 1.0000x reference)
"""Trainium2 Bass kernel for ConvTranspose3d(32->64, k=3, s=2, p=1) + inference
BatchNorm + per-(sample,channel) spatial mean subtraction.

Math: bias / beta / running_mean cancel exactly in the mean subtraction:
    out = A_c * (convT(x) - mean_spatial(convT(x))),  A_c = gamma/sqrt(var+eps)

Decomposition: stride-2 transpose conv -> 8 output parity classes.
Per dim, output o = 2j+p: p=0 uses kernel tap k=1 (input shift s=0);
p=1 uses taps k=2 (s=0) and k=0 (s=1).  The (sh,sw) shift variants of x are
baked into 4 partition groups of one SBUF tensor T1 (128 = 4x32ci partitions);
the d shift is a free-dim offset.  For each (ph,pw) class the two d-parities
pd=0/pd=1 are computed TOGETHER as psum partition halves (M=128 = 2x64 cout):
pass1 applies [W(kd=1) | W(kd=2)] to slab jd, pass2 accumulates [0 | W(kd=0)]
from slab jd+1.  16 matmul passes per jd instead of 24.

The per-pass weight tiles and the BN scale A are precomputed on the host and
loaded as single DMAs.  The spatial mean is computed analytically from 27 box
sums of x fed through the same weight tiles as 12 tiny matmuls (N=1), then
folded into the epilogue's per-partition scale+bias.

The four shifted group blocks of T1 are built by gpsimd casting DMAs
(f32 -> bf16) straight from HBM, with each (sh, sw) shift expressed as a
flat offset into x -- no SBUF-to-SBUF copies or vector casts at all.

Sharding: data-parallel, one sample per core (B=8, 8 cores).
"""

import numpy as np

B, CIN, COUT = 8, 32, 64
D, H, W = 16, 32, 32
DO, HO, WO = 31, 63, 63
EPS = 1e-5
NSPAT = DO * HO * WO

GROUPS = [(0, 0), (0, 1), (1, 0), (1, 1)]   # g = (sh, sw)
CLASSES = [(0, 0), (0, 1), (1, 0), (1, 1)]  # (ph, pw)
# (pass, col-half) -> kd tap: pass0 = slab jd: [kd1 | kd2]; pass1 = [ - | kd0]
PASS_KD = {(0, 0): 1, (0, 1): 2, (1, 1): 0}
NSLAB = 33 * 33          # padded (h, w) plane per d slab
NT1 = D * NSLAB
GOFF = [0, 1, 33, 34]    # flat shift per group (sh*33 + sw)


def _kmap(p, s):
    return 1 if p == 0 else (2 if s == 0 else 0)


def _rmap_kd(kd):
    # box range per dim given the kernel tap: Full / drop-Last / drop-fiRst
    return {1: 0, 2: 1, 0: 2}[kd]


def _tap_groups(ph, pw):
    return [gi for gi, (sh, sw) in enumerate(GROUPS)
            if not ((ph == 0 and sh != 0) or (pw == 0 and sw != 0))]


def build_nc():
    import concourse.bacc as bacc
    import concourse.mybir as mybir
    import concourse.tile as tile

    f32 = mybir.dt.float32
    bf16 = mybir.dt.bfloat16
    Alu = mybir.AluOpType
    Act = mybir.ActivationFunctionType

    nc = bacc.Bacc()
    x_d = nc.declare_dram_parameter("x", [CIN, D, H, W], f32, isOutput=False)
    wt_d = nc.declare_dram_parameter("wt", [128, 4, 2, 128], bf16, isOutput=False)
    wtm_d = nc.declare_dram_parameter("wtm", [128, 27, 128], bf16, isOutput=False)
    a_d = nc.declare_dram_parameter("arep", [128, 1], f32, isOutput=False)
    # plane-major output: each plane (or plane pair) is a fully contiguous
    # HBM write; the host transposes back to (co, d, h, w) afterwards
    o_d = nc.declare_dram_parameter("out", [DO, COUT, HO, WO], f32, isOutput=True)

    with tile.TileContext(nc) as tc:
        with (
            tc.tile_pool(name="singles", bufs=1) as singles,
            tc.tile_pool(name="stag", bufs=7) as stpool,
            tc.tile_pool(name="psum", bufs=8, space="PSUM") as pspool,
        ):
            # ---------------- PE warm-up (no data deps) ----------------
            dzero = singles.tile([128, 512], bf16)
            nc.vector.memset(dzero[:], 0.0)
            wps = pspool.tile([128, 512], f32, tag="main_ps")
            for r in range(16):
                nc.tensor.matmul(wps[:], dzero[:, 0:128], dzero[:],
                                 start=True, stop=True)

            # ---------------- loads ----------------
            # x for the mean path: one gpsimd CASTING load (f32 -> bf16),
            # 16KB contiguous runs; partition = (ci, dgrp).  It precedes the
            # T1 chunk loads on the gpsimd queue.
            packx = singles.tile([128, 4, H, W], bf16)
            xr = x_d[:].rearrange("c (g s) h w -> (c g) s h w", g=4)
            nc.gpsimd.dma_start(out=packx[:], in_=xr[:])
            # weight tiles + BN scale precomputed on host, single DMAs
            Wt = singles.tile([128, 4, 2, 128], bf16)
            nc.sync.dma_start(
                out=Wt[:].rearrange("p a b m -> p (a b m)"),
                in_=wt_d[:].rearrange("p a b m -> p (a b m)"))
            Wtm = singles.tile([128, 27, 128], bf16)
            nc.scalar.dma_start(
                out=Wtm[:].rearrange("p a m -> p (a m)"),
                in_=wtm_d[:].rearrange("p a m -> p (a m)"))
            arep = singles.tile([128, 1], f32)
            nc.scalar.dma_start(out=arep[:], in_=a_d[:])

            # ---------------- T1 build: casting loads from HBM ----------
            # Each 32-partition group block is loaded straight from HBM by a
            # gpsimd casting DMA (f32 -> bf16), with the (sh, sw) shift
            # expressed as a flat offset into x.  Wrap values at row/slab
            # boundaries land only in positions whose conv outputs the
            # epilogue trims (or that multiply zero weight rows).  Chunked
            # by 4-slab groups so early matmuls can start.
            T1 = singles.tile([128, D, H, W], bf16)
            T1f = T1[:].rearrange("p d h w -> p (d h w)")
            xflat = x_d[:].rearrange("c d h w -> c (d h w)")
            NCH = 4 * H * W                       # one 4-slab chunk
            for gi in range(1, 4):
                off = D * H * W - (32 * GROUPS[gi][0] + GROUPS[gi][1])
                nc.vector.memset(T1f[32 * gi:32 * gi + 32, off:], 0.0)
            for s in range(4):
                for gi in range(4):
                    sh, sw = GROUPS[gi]
                    off = 32 * sh + sw
                    cnt = NCH if s < 3 else NCH - off
                    nc.gpsimd.dma_start(
                        out=T1f[32 * gi:32 * gi + 32,
                                NCH * s:NCH * s + cnt],
                        in_=xflat[:, NCH * s + off:NCH * s + off + cnt])

            # ---------------- mean: per-slab box sums on 128 partitions --
            # Bt[(ci,dgrp), rh, rw, s] = box sum over (h-range, w-range) of
            # slab 4*dgrp+s; the d-range selection (full / drop d=15 /
            # drop d=0) is pushed into host-masked weight rows of the 27
            # tiny matmuls, so no partition gathers are needed at all.
            hsum = singles.tile([128, 4, H], f32)
            wc0 = singles.tile([128, 4, H], f32)
            wc31 = singles.tile([128, 4, H], f32)
            for s in range(4):
                nc.vector.reduce_sum(out=hsum[:, s:s + 1, :],
                                     in_=packx[:, s:s + 1],
                                     axis=mybir.AxisListType.X)
                nc.scalar.activation(out=wc0[:, s:s + 1, :],
                                     in_=packx[:, s:s + 1, :, 0:1].squeeze(3),
                                     func=Act.Copy, bias=0.0, scale=1.0)
                nc.scalar.activation(out=wc31[:, s:s + 1, :],
                                     in_=packx[:, s:s + 1, :, 31:32].squeeze(3),
                                     func=Act.Copy, bias=0.0, scale=1.0)
            rwL = singles.tile([128, 4, H], f32)
            rwR = singles.tile([128, 4, H], f32)
            nc.vector.tensor_sub(rwL[:], hsum[:], wc31[:])
            nc.vector.tensor_sub(rwR[:], hsum[:], wc0[:])

            Bt = singles.tile([128, 3, 3, 4], f32)
            for rw, t in ((0, hsum), (1, rwL), (2, rwR)):
                nc.vector.reduce_sum(
                    out=Bt[:, 0:1, rw:rw + 1, :].squeeze(2).squeeze(1),
                    in_=t[:], axis=mybir.AxisListType.X)
                hc0 = singles.tile([128, 4], f32, name=f"hc0_{rw}")
                hc31 = singles.tile([128, 4], f32, name=f"hc31_{rw}")
                nc.scalar.activation(out=hc0[:], in_=t[:, :, 0:1].squeeze(2),
                                     func=Act.Copy, bias=0.0, scale=1.0)
                nc.scalar.activation(out=hc31[:], in_=t[:, :, 31:32].squeeze(2),
                                     func=Act.Copy, bias=0.0, scale=1.0)
                nc.vector.tensor_sub(
                    Bt[:, 1:2, rw:rw + 1, :].squeeze(2).squeeze(1),
                    Bt[:, 0:1, rw:rw + 1, :].squeeze(2).squeeze(1), hc31[:])
                nc.vector.tensor_sub(
                    Bt[:, 2:3, rw:rw + 1, :].squeeze(2).squeeze(1),
                    Bt[:, 0:1, rw:rw + 1, :].squeeze(2).squeeze(1), hc0[:])
            Btb = singles.tile([128, 3, 3, 4], bf16)
            nc.vector.tensor_copy(out=Btb[:], in_=Bt[:])
            cfulf = singles.tile([128, 3, 3], f32)
            nc.vector.reduce_sum(out=cfulf[:], in_=Bt[:],
                                 axis=mybir.AxisListType.X)
            cful = singles.tile([128, 3, 3], bf16)
            nc.vector.tensor_copy(out=cful[:], in_=cfulf[:])

            # 27 tiny matmuls: 9 full-d + 2x9 host-masked boundary
            # corrections, accumulating the per-channel conv sum
            # M=128 with host-duplicated co columns: the mean lands on all
            # 128 psum partitions, so the bias never needs a replication DMA
            mps = pspool.tile([128, 512], f32, tag="main_ps")
            j = 0
            for v in range(3):
                for rh in range(3):
                    for rw in range(3):
                        if v == 0:
                            data = cful[0:128, rh, rw:rw + 1]
                        else:
                            sidx = 3 if v == 1 else 0
                            data = Btb[0:128, rh, rw, sidx:sidx + 1]
                        nc.tensor.matmul(
                            mps[0:128, 0:1], Wtm[0:128, j, :], data,
                            start=(j == 0), stop=(j == 26))
                        j += 1
            # bias = -A * mean  (all 128 partitions at once)
            msb = singles.tile([128, 1], f32)
            nc.scalar.activation(out=msb[:], in_=mps[0:128, 0:1],
                                 func=Act.Copy, bias=0.0, scale=1.0 / NSPAT)
            brep = singles.tile([128, 1], f32)
            nc.vector.tensor_scalar(out=brep[:], in0=msb[:],
                                    scalar1=arep[:], scalar2=-1.0,
                                    op0=Alu.mult, op1=Alu.mult)

            # ---------------- main loop ----------------
            epi = 0
            out_engs = [nc.sync, nc.scalar, nc.gpsimd]
            oeng_i = 0
            for jd in range(16):
                last = jd == 15          # only even output plane d=30
                stag = stpool.tile([128, HO, WO], f32)
                for nt in range(2):
                    for cls_i, (ph, pw) in enumerate(CLASSES):
                        ps = pspool.tile([128, 512], f32, tag="main_ps")
                        psv = ps[:].rearrange("p (a b) -> p a b", a=16)
                        # pass 1: slab jd with [W(kd1) | W(kd2)]
                        nc.tensor.matmul(
                            ps[:],
                            Wt[0:128, cls_i, 0, :],
                            T1[0:128, jd, 16 * nt:16 * nt + 16, 0:32],
                            start=True, stop=last)
                        if not last:
                            # pass 2: slab jd+1 with [0 | W(kd0)]
                            nc.tensor.matmul(
                                ps[:],
                                Wt[0:128, cls_i, 1, :],
                                T1[0:128, jd + 1, 16 * nt:16 * nt + 16, 0:32],
                                start=False, stop=True)
                        # epilogue: out = A*psum + bias, interleaved into plane
                        jhc = 16 if (ph == 0 or nt == 0) else 15
                        jwc = W - pw
                        np_ = 64 if last else 128
                        h0 = 32 * nt + ph
                        dest = stag[0:np_, h0:min(h0 + 2 * jhc, HO):2,
                                    pw:min(pw + 2 * jwc, WO):2]
                        srcv = psv[0:np_, 0:jhc, 0:jwc]
                        if epi % 2 == 0:
                            nc.scalar.activation(
                                out=dest, in_=srcv, func=Act.Identity,
                                bias=brep[0:np_, :], scale=arep[0:np_, :])
                        else:
                            nc.vector.tensor_scalar(
                                out=dest, in0=srcv,
                                scalar1=arep[0:np_, :], scalar2=brep[0:np_, :],
                                op0=Alu.mult, op1=Alu.add)
                        epi += 1
                # one contiguous 2MB output DMA per jd (both planes),
                # round-robin over all 3 queues
                eng = out_engs[oeng_i % 3]
                oeng_i += 1
                if last:
                    eng.dma_start(out=o_d[30:31], in_=stag[0:64])
                else:
                    eng.dma_start(out=o_d[2 * jd:2 * jd + 2], in_=stag[:])
    nc.compile()
    return nc


def _host_prep(inputs):
    import ml_dtypes
    x = np.ascontiguousarray(np.asarray(inputs["x"], dtype=np.float32))
    w = np.asarray(inputs["weight"], dtype=np.float32)
    # (ci, co, kd, kh, kw) -> (27 taps, ci, co)
    w27 = w.transpose(2, 3, 4, 0, 1).reshape(27, CIN, COUT)
    wt = np.zeros((128, 4, 2, 128), np.float32)   # rows = (g, ci) blocks
    for cls_i, (ph, pw) in enumerate(CLASSES):
        for (pi, half), kd in PASS_KD.items():
            for gi in _tap_groups(ph, pw):
                sh, sw = GROUPS[gi]
                kt = kd * 9 + _kmap(ph, sh) * 3 + _kmap(pw, sw)
                wt[32 * gi:32 * gi + 32, cls_i, pi,
                   64 * half:64 * half + 64] = w27[kt]
    wt = np.ascontiguousarray(wt.astype(ml_dtypes.bfloat16))
    # mean-path weights: 27 columns = 9 full-d (summed over kd) + 9 "drop
    # d=15" corrections (kd=2 tap, only dgrp=3 rows) + 9 "drop d=0"
    # corrections (kd=0 tap, only dgrp=0 rows).  Rows = (ci, dgrp) to match
    # packx partitions.  KH/KW invert _rmap_kd: box range r -> kernel tap.
    KINV = {0: 1, 1: 2, 2: 0}
    wtm = np.zeros((CIN, 4, 27, COUT), np.float32)
    for rh in range(3):
        for rw in range(3):
            kh, kw = KINV[rh], KINV[rw]
            main = sum(w27[kd * 9 + kh * 3 + kw] for kd in range(3))
            wtm[:, :, 0 * 9 + rh * 3 + rw, :] = main[:, None, :]
            wtm[:, 3, 1 * 9 + rh * 3 + rw, :] = -w27[2 * 9 + kh * 3 + kw]
            wtm[:, 0, 2 * 9 + rh * 3 + rw, :] = -w27[0 * 9 + kh * 3 + kw]
    wtm = np.concatenate([wtm, wtm], axis=3)   # duplicate co -> M=128
    wtm = np.ascontiguousarray(
        wtm.reshape(128, 27, 128).astype(ml_dtypes.bfloat16))
    gamma = np.asarray(inputs["gamma"], dtype=np.float32)
    rvar = np.asarray(inputs["running_var"], dtype=np.float32)
    a = gamma / np.sqrt(rvar + EPS)
    arep = np.ascontiguousarray(
        np.concatenate([a, a]).reshape(128, 1).astype(np.float32))
    return x, wt, wtm, arep


def run(inputs, trace=False):
    from concourse.bass_utils import run_bass_kernel_spmd

    nc = _get_nc()
    x, wt, wtm, arep = _host_prep(inputs)
    in_maps = [{"x": x[k], "wt": wt, "wtm": wtm, "arep": arep}
               for k in range(B)]
    res = run_bass_kernel_spmd(nc, in_maps, core_ids=list(range(B)), trace=trace)
    out = np.stack([res.results[k]["out"].transpose(1, 0, 2, 3)
                    for k in range(B)], axis=0)
    return out, res


_NC = None


def _get_nc():
    global _NC
    if _NC is None:
        _NC = build_nc()
    return _NC


def kernel(**inputs) -> np.ndarray:
    out, _ = run(inputs, trace=False)
    return out


# ---------------------------------------------------------------------------
# Benchmarking helpers (test.py only; the grader uses kernel() above).
# ---------------------------------------------------------------------------

def enable_axon_profiling():
    """Register the missing antenv.axon_hooks shim so that
    run_bass_kernel_spmd(trace=True) can capture NTFF profiles through the
    axon PJRT .so (see trn_agent_boot.trn_boot)."""
    import sys
    import types
    try:
        import antenv.axon_hooks  # noqa: F401
        return True
    except ImportError:
        pass
    mod = types.ModuleType("antenv.axon_hooks")
    mod._hook = None

    def set_axon_ntff_profile_hook(h):
        mod._hook = h

    def get_axon_ntff_profile_hook():
        return mod._hook

    mod.set_axon_ntff_profile_hook = set_axon_ntff_profile_hook
    mod.get_axon_ntff_profile_hook = get_axon_ntff_profile_hook
    sys.modules["antenv.axon_hooks"] = mod
    import antenv
    antenv.axon_hooks = mod
    from trn_agent_boot.trn_boot import _ntff_profile_via_ctypes
    hook = _ntff_profile_via_ctypes('/opt/axon/libaxon_pjrt.so')
    if hook is None:
        return False
    mod._hook = hook
    return True

def _build_sharded_fn(nc, n_cores=B):
    import jax
    from jax.experimental.shard_map import shard_map
    from jax.sharding import Mesh, PartitionSpec
    import concourse.mybir as mybir
    from concourse import bass2jax

    bass2jax.install_neuronx_cc_hook()
    partition_name = (nc.partition_id_tensor.name
                      if nc.partition_id_tensor else None)
    in_names, out_names, out_avals, zero_outs = [], [], [], []
    for alloc in nc.m.functions[0].allocations:
        if not isinstance(alloc, mybir.MemoryLocationSet):
            continue
        name = alloc.memorylocations[0].name
        if alloc.kind == "ExternalInput":
            if name != partition_name:
                in_names.append(name)
        elif alloc.kind == "ExternalOutput":
            out_names.append(name)
            shape = tuple(alloc.tensor_shape)
            dtype = mybir.dt.np(alloc.dtype)
            out_avals.append(jax.core.ShapedArray(shape, dtype))
            zero_outs.append(np.zeros(shape, dtype))
    n_params = len(in_names)
    all_names = in_names + out_names
    if partition_name is not None:
        all_names = all_names + [partition_name]
    donate = tuple(range(n_params, n_params + len(out_names)))

    def _body(*args):
        operands = list(args)
        if partition_name is not None:
            operands.append(bass2jax.partition_id_tensor())
        outs = bass2jax._bass_exec_p.bind(
            *operands,
            out_avals=tuple(out_avals),
            in_names=tuple(all_names),
            out_names=tuple(out_names),
            lowering_input_output_aliases=(),
            sim_require_finite=True,
            sim_require_nnan=True,
            nc=nc,
        )
        return tuple(outs)

    devices = jax.devices()[:n_cores]
    mesh = Mesh(np.asarray(devices), ("core",))
    nspec = (PartitionSpec("core"),)
    fn = jax.jit(
        shard_map(_body, mesh=mesh, in_specs=nspec * (n_params + len(out_names)),
                  out_specs=nspec * len(out_names), check_rep=False),
        donate_argnums=donate, keep_unused=True)
    return fn, mesh, in_names, out_names, out_avals, zero_outs


def _build_null_nc():
    import concourse.bacc as bacc
    import concourse.mybir as mybir
    import concourse.tile as tile

    f32 = mybir.dt.float32
    nc = bacc.Bacc()
    a = nc.declare_dram_parameter("a", [1, 32], f32, isOutput=False)
    bout = nc.declare_dram_parameter("b", [1, 32], f32, isOutput=True)
    with tile.TileContext(nc) as tc:
        with tc.tile_pool(name="p", bufs=1) as pool:
            t = pool.tile([1, 32], f32)
            nc.sync.dma_start(out=t[:], in_=a[:])
            nc.sync.dma_start(out=bout[:], in_=t[:])
    nc.compile()
    return nc


def _bench_nc(nc, per_core_inputs, iters):
    """per_core_inputs: list over cores of dict name->array. Returns
    (list of wall seconds, outputs of last iter as list over cores)."""
    import time
    import jax
    from jax.sharding import NamedSharding, PartitionSpec

    n_cores = len(per_core_inputs)
    fn, mesh, in_names, out_names, out_avals, zero_outs = _build_sharded_fn(
        nc, n_cores)
    sh = NamedSharding(mesh, PartitionSpec("core"))
    in_dev = [
        jax.device_put(
            np.concatenate([np.asarray(per_core_inputs[c][n])
                            for c in range(n_cores)], axis=0), sh)
        for n in in_names
    ]

    def fresh_zeros():
        return [jax.device_put(
            np.zeros((n_cores * z.shape[0], *z.shape[1:]), z.dtype), sh)
            for z in zero_outs]

    # warmup (compile)
    outs = fn(*in_dev, *fresh_zeros())
    jax.block_until_ready(outs)
    times = []
    for _ in range(iters):
        zs = fresh_zeros()
        jax.block_until_ready(zs)
        t0 = time.perf_counter()
        outs = fn(*in_dev, *zs)
        jax.block_until_ready(outs)
        times.append(time.perf_counter() - t0)
    res = [
        {n: np.asarray(outs[i]).reshape(n_cores, *out_avals[i].shape)[c]
         for i, n in enumerate(out_names)}
        for c in range(n_cores)
    ]
    return times, res


def bench(inputs, iters=8):
    x, wt, wtm, arep = _host_prep(inputs)
    per_core = [{"x": x[k], "wt": wt, "wtm": wtm, "arep": arep}
                for k in range(B)]
    times, res = _bench_nc(_get_nc(), per_core, iters)
    out = np.stack([res[k]["out"].transpose(1, 0, 2, 3) for k in range(B)],
                   axis=0)

    null_per_core = [{"a": np.zeros((1, 32), np.float32)} for _ in range(B)]
    null_times, _ = _bench_nc(_build_null_nc(), null_per_core, iters)
    return out, times, null_times



# revision 7
# speedup vs baseline: 1.5949x; 1.5949x over previous
"""Trainium2 Bass kernel for ConvTranspose3d(32->64, k=3, s=2, p=1) + inference
BatchNorm + per-(sample,channel) spatial mean subtraction.

Math: bias / beta / running_mean cancel exactly in the mean subtraction:
    out = A_c * (convT(x) - mean_spatial(convT(x))),  A_c = gamma/sqrt(var+eps)
A_c is folded into the conv weights on the host, so the device epilogue is a
single per-partition bias add (bias = -A*mean).

Decomposition: stride-2 transpose conv -> 8 output parity classes.
Per dim, output o = 2j+p: p=0 uses kernel tap k=1 (input shift s=0);
p=1 uses taps k=2 (s=0) and k=0 (s=1).  The (sh,sw) shift variants of x are
baked into 4 partition groups of one SBUF tensor T1 (128 = 4x32ci partitions);
the d shift is a free-dim offset.  Unlike the 8-pass (2 d-parities as psum
halves) scheme, psum M-halves pack two (ph,pw) CLASSES at the same d-parity:
  tile A/B: [c(1,1) | c(1,0)] at pd=1 / pd=0
  tile C/D: [c(0,1) | c(0,0)] at pd=1 / pd=0
pd=1 tiles take 2 passes (kd=2 @ slab jd, kd=0 @ slab jd+1), pd=0 tiles take
1 pass (kd=1 @ jd): 6 matmul passes per (jd, nt) instead of 8.

HBM traffic minimized: x is read once for T1 (gpsimd casting DMA f32->bf16,
group 0 only); groups 1-3 are SBUF->SBUF DMA copies with a flat shift offset.
The output is written as bf16 in a packed class-blocked layout
[jd, part, nt, tile, 512] (fully contiguous epilogue writes + contiguous
1MB DMAs); the host de-interleaves the parity classes and upcasts to f32.

The spatial mean is computed analytically from 27 box sums of x fed through
the same (A-scaled) weight taps as 27 tiny matmuls (N=1), giving the
epilogue's per-partition bias directly.

Sharding: data-parallel, one sample per core (B=8, 8 cores).
"""

import numpy as np

B, CIN, COUT = 8, 32, 64
D, H, W = 16, 32, 32
DO, HO, WO = 31, 63, 63
EPS = 1e-5
NSPAT = DO * HO * WO

GROUPS = [(0, 0), (0, 1), (1, 0), (1, 1)]   # g = (sh, sw)
# tile kinds: ((class in M half0, class in M half1), d-parity)
TILES = [
    (((1, 1), (1, 0)), 1),   # A: odd plane,  passes kd2 @ jd, kd0 @ jd+1
    (((1, 1), (1, 0)), 0),   # B: even plane, pass  kd1 @ jd
    (((0, 1), (0, 0)), 1),   # C
    (((0, 1), (0, 0)), 0),   # D
]
NCH = 4 * H * W          # one 4-slab chunk of T1's free dim
NT1 = D * H * W


def _kmap(p, s):
    return 1 if p == 0 else (2 if s == 0 else 0)


def _tap_groups(ph, pw):
    return [gi for gi, (sh, sw) in enumerate(GROUPS)
            if not ((ph == 0 and sh != 0) or (pw == 0 and sw != 0))]


def build_nc():
    import concourse.bacc as bacc
    import concourse.mybir as mybir
    import concourse.tile as tile

    f32 = mybir.dt.float32
    bf16 = mybir.dt.bfloat16
    Alu = mybir.AluOpType
    Act = mybir.ActivationFunctionType

    nc = bacc.Bacc()
    x_d = nc.declare_dram_parameter("x", [CIN, D, H, W], f32, isOutput=False)
    wt_d = nc.declare_dram_parameter("wt", [128, 4, 2, 128], bf16, isOutput=False)
    wtm_d = nc.declare_dram_parameter("wtm", [128, 27, 128], bf16, isOutput=False)
    # packed class-blocked bf16 output: host de-interleaves + upcasts
    o_d = nc.declare_dram_parameter("out", [16, 128, 2, 4, 512], bf16,
                                    isOutput=True)

    with tile.TileContext(nc) as tc:
        with (
            tc.tile_pool(name="singles", bufs=1) as singles,
            tc.tile_pool(name="stag", bufs=4) as stpool,
            tc.tile_pool(name="psum", bufs=8, space="PSUM") as pspool,
        ):
            # ---------------- PE warm-up (no data deps) ----------------
            dzero = singles.tile([128, 512], bf16)
            nc.vector.memset(dzero[:], 0.0)
            wps = pspool.tile([128, 512], f32, tag="main_ps")
            for r in range(16):
                nc.tensor.matmul(wps[:], dzero[:, 0:128], dzero[:],
                                 start=True, stop=True)

            # ---------------- loads ----------------
            # x for the mean path: one gpsimd CASTING load (f32 -> bf16),
            # partition = (ci, dgrp).  Precedes the T1 loads on gpsimd.
            packx = singles.tile([128, 4, H, W], bf16)
            xr = x_d[:].rearrange("c (g s) h w -> (c g) s h w", g=4)
            nc.gpsimd.dma_start(out=packx[:], in_=xr[:])
            # weight tiles precomputed on host (BN scale folded in)
            Wt = singles.tile([128, 4, 2, 128], bf16)
            nc.sync.dma_start(
                out=Wt[:].rearrange("p a b m -> p (a b m)"),
                in_=wt_d[:].rearrange("p a b m -> p (a b m)"))
            Wtm = singles.tile([128, 27, 128], bf16)
            nc.scalar.dma_start(
                out=Wtm[:].rearrange("p a m -> p (a m)"),
                in_=wtm_d[:].rearrange("p a m -> p (a m)"))

            # ---------------- T1 build ----------
            # Group 0 (unshifted x) is loaded from HBM by gpsimd casting DMAs
            # (f32 -> bf16) in 4-slab chunks; groups 1-3 are SBUF->SBUF DMA
            # copies of group 0 with the (sh, sw) shift as a flat offset.
            # Wrap values at row/slab boundaries land only in positions whose
            # conv outputs the host trims (or that multiply zero weights).
            # Chunk-s copies read `off` elements into chunk s+1, so they are
            # issued after the s+1 group-0 load (Tile tracks the overlap).
            T1 = singles.tile([128, D, H, W], bf16)
            T1f = T1[:].rearrange("p d h w -> p (d h w)")
            xflat = x_d[:].rearrange("c d h w -> c (d h w)")
            for gi in range(1, 4):
                off = NT1 - (32 * GROUPS[gi][0] + GROUPS[gi][1])
                nc.vector.memset(T1f[32 * gi:32 * gi + 32, off:], 0.0)
            copy_engs = [nc.sync, nc.scalar, nc.sync]
            for s in range(4):
                nc.gpsimd.dma_start(
                    out=T1f[0:32, NCH * s:NCH * (s + 1)],
                    in_=xflat[:, NCH * s:NCH * (s + 1)])
            for s in range(4):
                for gi in range(1, 4):
                    sh, sw = GROUPS[gi]
                    off = 32 * sh + sw
                    cnt = NCH if s < 3 else NCH - off
                    copy_engs[gi - 1].dma_start(
                        out=T1f[32 * gi:32 * gi + 32, NCH * s:NCH * s + cnt],
                        in_=T1f[0:32, NCH * s + off:NCH * s + off + cnt])

            # ---------------- mean: per-slab box sums on 128 partitions --
            # Bt[(ci,dgrp), rh, rw, s] = box sum over (h-range, w-range) of
            # slab 4*dgrp+s; the d-range selection (full / drop d=15 /
            # drop d=0) is pushed into host-masked weight rows of the 27
            # tiny matmuls, so no partition gathers are needed at all.
            hsum = singles.tile([128, 4, H], f32)
            wc0 = singles.tile([128, 4, H], f32)
            wc31 = singles.tile([128, 4, H], f32)
            for s in range(4):
                nc.vector.reduce_sum(out=hsum[:, s:s + 1, :],
                                     in_=packx[:, s:s + 1],
                                     axis=mybir.AxisListType.X)
                nc.scalar.activation(out=wc0[:, s:s + 1, :],
                                     in_=packx[:, s:s + 1, :, 0:1].squeeze(3),
                                     func=Act.Copy, bias=0.0, scale=1.0)
                nc.scalar.activation(out=wc31[:, s:s + 1, :],
                                     in_=packx[:, s:s + 1, :, 31:32].squeeze(3),
                                     func=Act.Copy, bias=0.0, scale=1.0)
            rwL = singles.tile([128, 4, H], f32)
            rwR = singles.tile([128, 4, H], f32)
            nc.vector.tensor_sub(rwL[:], hsum[:], wc31[:])
            nc.vector.tensor_sub(rwR[:], hsum[:], wc0[:])

            Bt = singles.tile([128, 3, 3, 4], f32)
            for rw, t in ((0, hsum), (1, rwL), (2, rwR)):
                nc.vector.reduce_sum(
                    out=Bt[:, 0:1, rw:rw + 1, :].squeeze(2).squeeze(1),
                    in_=t[:], axis=mybir.AxisListType.X)
                hc0 = singles.tile([128, 4], f32, name=f"hc0_{rw}")
                hc31 = singles.tile([128, 4], f32, name=f"hc31_{rw}")
                nc.scalar.activation(out=hc0[:], in_=t[:, :, 0:1].squeeze(2),
                                     func=Act.Copy, bias=0.0, scale=1.0)
                nc.scalar.activation(out=hc31[:], in_=t[:, :, 31:32].squeeze(2),
                                     func=Act.Copy, bias=0.0, scale=1.0)
                nc.vector.tensor_sub(
                    Bt[:, 1:2, rw:rw + 1, :].squeeze(2).squeeze(1),
                    Bt[:, 0:1, rw:rw + 1, :].squeeze(2).squeeze(1), hc31[:])
                nc.vector.tensor_sub(
                    Bt[:, 2:3, rw:rw + 1, :].squeeze(2).squeeze(1),
                    Bt[:, 0:1, rw:rw + 1, :].squeeze(2).squeeze(1), hc0[:])
            Btb = singles.tile([128, 3, 3, 4], bf16)
            nc.vector.tensor_copy(out=Btb[:], in_=Bt[:])
            cfulf = singles.tile([128, 3, 3], f32)
            nc.vector.reduce_sum(out=cfulf[:], in_=Bt[:],
                                 axis=mybir.AxisListType.X)
            cful = singles.tile([128, 3, 3], bf16)
            nc.vector.tensor_copy(out=cful[:], in_=cfulf[:])

            # 27 tiny matmuls: 9 full-d + 2x9 host-masked boundary
            # corrections, accumulating the per-channel (A-scaled) conv sum.
            # M=128 with host-duplicated co columns: the mean lands on all
            # 128 psum partitions, so the bias never needs a replication DMA.
            mps = pspool.tile([128, 512], f32, tag="main_ps")
            j = 0
            for v in range(3):
                for rh in range(3):
                    for rw in range(3):
                        if v == 0:
                            data = cful[0:128, rh, rw:rw + 1]
                        else:
                            sidx = 3 if v == 1 else 0
                            data = Btb[0:128, rh, rw, sidx:sidx + 1]
                        nc.tensor.matmul(
                            mps[0:128, 0:1], Wtm[0:128, j, :], data,
                            start=(j == 0), stop=(j == 26))
                        j += 1
            # bias = -(A*mean)  (all 128 partitions at once)
            brep = singles.tile([128, 1], f32)
            nc.scalar.activation(out=brep[:], in_=mps[0:128, 0:1],
                                 func=Act.Copy, bias=0.0, scale=-1.0 / NSPAT)

            # ---------------- main loop ----------------
            epi_engs = [nc.vector, nc.scalar]
            out_engs = [nc.sync, nc.scalar]
            epi = 0
            for jd in range(16):
                last = jd == 15
                stag = stpool.tile([128, 2, 4, 512], bf16)
                if last:
                    # odd-plane tiles (A, C) don't exist for d=31
                    nc.vector.memset(stag[:, :, 0, :], 0.0)
                    nc.vector.memset(stag[:, :, 2, :], 0.0)
                for nt in range(2):
                    for t, (_, pd) in enumerate(TILES):
                        if last and pd == 1:
                            continue
                        ps = pspool.tile([128, 512], f32, tag="main_ps")
                        rhs0 = T1[0:128, jd, 16 * nt:16 * nt + 16, 0:32]
                        if pd == 0:
                            nc.tensor.matmul(ps[:], Wt[0:128, t, 0, :], rhs0,
                                             start=True, stop=True)
                        else:
                            nc.tensor.matmul(ps[:], Wt[0:128, t, 0, :], rhs0,
                                             start=True, stop=False)
                            rhs1 = T1[0:128, jd + 1, 16 * nt:16 * nt + 16, 0:32]
                            nc.tensor.matmul(ps[:], Wt[0:128, t, 1, :], rhs1,
                                             start=False, stop=True)
                        # epilogue: out = psum + bias, contiguous bf16 write
                        dest = stag[0:128, nt, t, :]
                        eng = epi_engs[epi % 2]
                        epi += 1
                        if eng is nc.scalar:
                            nc.scalar.activation(
                                out=dest, in_=ps[:], func=Act.Identity,
                                bias=brep[0:128, :], scale=1.0)
                        else:
                            eng.tensor_scalar(
                                out=dest, in0=ps[:], scalar1=brep[0:128, :],
                                scalar2=None, op0=Alu.add)
                # one contiguous 1MB output DMA per jd
                out_engs[jd % 2].dma_start(out=o_d[jd:jd + 1], in_=stag[:])
    nc.compile()
    return nc


def _host_prep(inputs):
    import ml_dtypes
    x = np.ascontiguousarray(np.asarray(inputs["x"], dtype=np.float32))
    w = np.asarray(inputs["weight"], dtype=np.float32)
    gamma = np.asarray(inputs["gamma"], dtype=np.float32)
    rvar = np.asarray(inputs["running_var"], dtype=np.float32)
    a = gamma / np.sqrt(rvar + EPS)
    # (ci, co, kd, kh, kw) -> (27 taps, ci, co), BN scale folded in
    w27 = w.transpose(2, 3, 4, 0, 1).reshape(27, CIN, COUT) * a[None, None, :]
    wt = np.zeros((128, 4, 2, 128), np.float32)   # rows = (g, ci) blocks
    for t, ((cA, cB), pd) in enumerate(TILES):
        for half, (ph, pw) in enumerate((cA, cB)):
            for p in range(2):
                if pd == 0 and p == 1:
                    continue
                kd = 1 if pd == 0 else (2 if p == 0 else 0)
                for gi in _tap_groups(ph, pw):
                    sh, sw = GROUPS[gi]
                    kt = kd * 9 + _kmap(ph, sh) * 3 + _kmap(pw, sw)
                    wt[32 * gi:32 * gi + 32, t, p,
                       64 * half:64 * half + 64] = w27[kt]
    wt = np.ascontiguousarray(wt.astype(ml_dtypes.bfloat16))
    # mean-path weights: 27 columns = 9 full-d (summed over kd) + 9 "drop
    # d=15" corrections (kd=2 tap, only dgrp=3 rows) + 9 "drop d=0"
    # corrections (kd=0 tap, only dgrp=0 rows).  Rows = (ci, dgrp) to match
    # packx partitions.  KH/KW invert the box-range map: r -> kernel tap.
    KINV = {0: 1, 1: 2, 2: 0}
    wtm = np.zeros((CIN, 4, 27, COUT), np.float32)
    for rh in range(3):
        for rw in range(3):
            kh, kw = KINV[rh], KINV[rw]
            main = sum(w27[kd * 9 + kh * 3 + kw] for kd in range(3))
            wtm[:, :, 0 * 9 + rh * 3 + rw, :] = main[:, None, :]
            wtm[:, 3, 1 * 9 + rh * 3 + rw, :] = -w27[2 * 9 + kh * 3 + kw]
            wtm[:, 0, 2 * 9 + rh * 3 + rw, :] = -w27[0 * 9 + kh * 3 + kw]
    wtm = np.concatenate([wtm, wtm], axis=3)   # duplicate co -> M=128
    wtm = np.ascontiguousarray(
        wtm.reshape(128, 27, 128).astype(ml_dtypes.bfloat16))
    return x, wt, wtm


def _decode(o8):
    """(16, 128, 2, 4, 512) bf16 device output -> (COUT, DO, HO, WO) f32."""
    o = np.asarray(o8).astype(np.float32).reshape(16, 128, 2, 4, 16, 32)
    out = np.empty((COUT, DO, HO, WO), np.float32)
    for t, ((cA, cB), pd) in enumerate(TILES):
        for half, (ph, pw) in enumerate((cA, cB)):
            arr = o[:, 64 * half:64 * half + 64, :, t]   # (16, 64, 2, 16, 32)
            arr = arr.transpose(1, 0, 2, 3, 4).reshape(COUT, 16, 32, 32)
            nd = 16 if pd == 0 else 15
            nh = 32 if ph == 0 else 31
            nw = 32 if pw == 0 else 31
            out[:, pd::2, ph::2, pw::2] = arr[:, :nd, :nh, :nw]
    return out


def run(inputs, trace=False):
    from concourse.bass_utils import run_bass_kernel_spmd

    nc = _get_nc()
    x, wt, wtm = _host_prep(inputs)
    in_maps = [{"x": x[k], "wt": wt, "wtm": wtm} for k in range(B)]
    res = run_bass_kernel_spmd(nc, in_maps, core_ids=list(range(B)), trace=trace)
    out = np.stack([_decode(res.results[k]["out"]) for k in range(B)], axis=0)
    return out, res


_NC = None


def _get_nc():
    global _NC
    if _NC is None:
        _NC = build_nc()
    return _NC


def kernel(**inputs) -> np.ndarray:
    out, _ = run(inputs, trace=False)
    return out


# ---------------------------------------------------------------------------
# Benchmarking helpers (test.py only; the grader uses kernel() above).
# ---------------------------------------------------------------------------

def enable_axon_profiling():
    """Register the missing antenv.axon_hooks shim so that
    run_bass_kernel_spmd(trace=True) can capture NTFF profiles through the
    axon PJRT .so (see trn_agent_boot.trn_boot)."""
    import sys
    import types
    try:
        import antenv.axon_hooks  # noqa: F401
        return True
    except ImportError:
        pass
    mod = types.ModuleType("antenv.axon_hooks")
    mod._hook = None

    def set_axon_ntff_profile_hook(h):
        mod._hook = h

    def get_axon_ntff_profile_hook():
        return mod._hook

    mod.set_axon_ntff_profile_hook = set_axon_ntff_profile_hook
    mod.get_axon_ntff_profile_hook = get_axon_ntff_profile_hook
    sys.modules["antenv.axon_hooks"] = mod
    import antenv
    antenv.axon_hooks = mod
    from trn_agent_boot.trn_boot import _ntff_profile_via_ctypes
    hook = _ntff_profile_via_ctypes('/opt/axon/libaxon_pjrt.so')
    if hook is None:
        return False
    mod._hook = hook
    return True
